# revision 138
# baseline (speedup 1.0000x reference)
"""Causal self-attention with RoPE, tensor-parallel over heads on 8 trn2 cores.

Reference computation (B=1, T=4096, C=1024, h=16, d=64, fp32):
    q/k/v = x @ W{q,k,v}^T ; rope(q), rope(k) ; causal softmax(q k^T / 8) v ; @ Wo^T

Sharding: 2 heads per core (tensor parallel). Each core reads the full x
(transposed + bf16 on host) and its slice of Wq/Wk/Wv (column-parallel) and
Wo (row-parallel). Cores emit partial o-projections; the host sums them.

Device-side layout choices:
  - qT/kT [dhead(=128 both heads) x T] with the head dim de-interleaved
    (rope real parts in partitions 0-31 / 64-95, imag in 32-63 / 96-127) so
    rope's pair-swap is a partition-block swap done by 4 small DMAs.
  - scores are computed transposed: sT[j, i] = sum_d kT[d,j] qT[d,i], so the
    softmax normalizer is a sum over PARTITIONS, obtained for free by
    augmenting v with a ones column in the att @ v matmul (row 64 of the
    y-psum accumulates the denominator).
  - v is produced transposed like q/k then PE-transposed to natural [t, d]
    blocks (needed as the stationary operand of the att@v matmul).
  - causal masking: only diagonal j-tiles need masking; 4 static [128,512]
    masks (one per 128-offset within a 512 column group) multiply exp'd
    scores. Fully-masked subtiles are skipped in the att@v accumulation.
"""

import numpy as np
import ml_dtypes

bf16 = ml_dtypes.bfloat16
fp8 = ml_dtypes.float8_e4m3
WSCALE = 32.0  # qkv weights host-scaled into fp8's normal range

T, C, H, D = 4096, 1024, 16, 64
NCORES = 8
HPC = H // NCORES          # heads per core
DD = HPC * D               # per-core qkv features (=128)
P = 128

_nc_cache = {}


def _build_nc(t=T):
    import concourse.bass as bass
    import concourse.tile as tile
    import concourse.mybir as mybir
    from concourse import bacc
    from concourse.masks import make_identity

    f32 = mybir.dt.float32
    b16 = mybir.dt.bfloat16
    f8 = mybir.dt.float8e4
    DR = mybir.MatmulPerfMode.DoubleRow
    MUL = mybir.AluOpType.mult
    EXP = mybir.ActivationFunctionType.Exp

    nt = t // 512            # qkv t-chunks
    nw = t // 1024           # attention query windows
    njb = t // P             # key blocks

    nc = bacc.Bacc("TRN2")

    xt_d = nc.dram_tensor("xt", [C, t], b16, kind="ExternalInput")
    psw_d = nc.dram_tensor("permsw", [P, P], b16, kind="ExternalInput")
    # weights pre-laid-out on host as [p, co, m] so the load is contiguous
    wq_d = nc.dram_tensor("wq", [P, C // P, DD], b16, kind="ExternalInput")
    wk_d = nc.dram_tensor("wk", [P, C // P, DD], b16, kind="ExternalInput")
    wv_d = nc.dram_tensor("wv", [P, C // P, DD], b16, kind="ExternalInput")
    wo_d = nc.dram_tensor("wo", [DD, C], b16, kind="ExternalInput")
    cos_d = nc.dram_tensor("cosb", [P, t], b16, kind="ExternalInput")
    sin_d = nc.dram_tensor("sinb", [P, t], b16, kind="ExternalInput")
    msk_d = nc.dram_tensor("mask4", [P, 4, 512], b16, kind="ExternalInput")
    out_d = nc.dram_tensor("opart", [t, C], b16, kind="ExternalOutput")

    with tile.TileContext(nc) as tc:
        with (
            tc.tile_pool(name="const", bufs=1) as constp,
            tc.tile_pool(name="xload", bufs=3) as xload,
            tc.tile_pool(name="rope", bufs=3) as ropep,
            tc.tile_pool(name="att", bufs=6) as attp,
            tc.tile_pool(name="small", bufs=4) as smallp,
        ):
            # ---- constants / persistent tensors. wq then x-chunk 0 go first
            # so the first matmuls aren't stuck behind the other const loads.
            xts = {}

            def x_load(tch):
                tsl = slice(tch * 512, (tch + 1) * 512)
                xt = xload.tile([P, C // P, 512], b16, name="xt")
                nc.sync.dma_start(
                    xt, xt_d[:].rearrange("(co p) t -> p co t", p=P)[:, :, tsl]
                )
                xts[tch] = xt

            # load order is latency-tuned: chunk 0/1 inputs and the rope/mask
            # tables for the first window come before the bulk cos/sin cols.
            wq_sb = constp.tile([P, C // P, DD], b16)
            nc.sync.dma_start(wq_sb, wq_d[:])
            x_load(0)
            wk_sb = constp.tile([P, C // P, DD], b16)
            nc.sync.dma_start(wk_sb, wk_d[:])
            wv_sb = constp.tile([P, C // P, DD], b16)
            nc.sync.dma_start(wv_sb, wv_d[:])
            # rope-table slices are interleaved with the early x chunks so
            # neither the first windows nor chunks 2/3 wait on bulk loads:
            # each chunk tch only needs cos/sin cols [512*tch, 512*tch+512)
            cos_sb = constp.tile([P, t], b16)
            sin_sb = constp.tile([P, t], b16)
            c1 = min(512, t)
            c2 = min(1024, t)
            nc.sync.dma_start(cos_sb[:, 0:c1], cos_d[:, 0:c1])
            nc.sync.dma_start(sin_sb[:, 0:c1], sin_d[:, 0:c1])
            ident = constp.tile([P, P], b16)
            make_identity(nc, ident)
            psw_sb = constp.tile([P, P], b16)
            nc.sync.dma_start(psw_sb, psw_d[:])
            if nt > 1:
                x_load(1)
            msk_sb = constp.tile([P, 4, 512], b16)
            nc.sync.dma_start(msk_sb, msk_d[:])
            if c2 > c1:
                nc.sync.dma_start(cos_sb[:, c1:c2], cos_d[:, c1:c2])
                nc.sync.dma_start(sin_sb[:, c1:c2], sin_d[:, c1:c2])
            if nt > 2:
                x_load(2)
            if t > c2:
                nc.sync.dma_start(cos_sb[:, c2:], cos_d[:, c2:])
                nc.sync.dma_start(sin_sb[:, c2:], sin_d[:, c2:])
            wo_sb = constp.tile([DD, C], b16)
            nc.sync.dma_start(wo_sb, wo_d[:])

            qT = constp.tile([P, t], b16)   # rope'd q, both heads
            kT = constp.tile([P, t], b16)
            yT = constp.tile([P, t], b16)   # normalized attention output
            # v in natural layout per 128-block, +ones cols at 64 and 129
            vaug = constp.tile([P, njb, 2 * D + 2], b16)
            nc.vector.memset(vaug[:, :, D], 1.0)
            nc.vector.memset(vaug[:, :, 2 * D + 1], 1.0)

            # ---- phase 1: qkv projections + rope + v transpose,
            # with the first two 512-wide attention windows interleaved so
            # the ACT engine starts exp work while qkv is still streaming.
            # PSUM: ph1 drains 4 banks (bufs=1) + early-attention 4 banks.
            with (
                tc.tile_pool(name="psqkv", bufs=1, space="PSUM") as psqkv,
                tc.tile_pool(name="psearly", bufs=1, space="PSUM") as psearly,
            ):
                vts = {}

                def v_transposes(tch):
                    vt = vts.pop(tch)
                    for tb in range(4):
                        pst = psqkv.tile([P, P], b16, tag="pst", name="pst")
                        nc.tensor.transpose(pst, vt[:, tb * P:(tb + 1) * P], ident)
                        g = tch * 4 + tb
                        nc.vector.tensor_copy(vaug[:, g, 0:D], pst[:, 0:D])
                        nc.vector.tensor_copy(vaug[:, g, D + 1:2 * D + 1],
                                              pst[:, D:2 * D])

                # phase-1 attention work queue: 512-wide windows over query
                # rows 0..2048, chopped into per-(jc) items that are emitted
                # BETWEEN projection groups. The PE queue is in-order, so an
                # att@v matmul waiting on its exp would stall every later
                # chunk matmul behind it; fine slicing keeps each item's exp
                # ~2 projection-groups ahead of its consumer.
                win_items = []
                psyE = {}

                def push_window(iw):
                    njc = 4 * (iw + 1)
                    for jc in range(njc):
                        win_items.append((iw, jc, njc))

                def emit_items(n):
                    for _ in range(min(n, len(win_items))):
                        iw, jc, njc = win_items.pop(0)
                        jsl = slice(jc * P, (jc + 1) * P)
                        mb = jc - 4 * iw
                        # window-local columns < 128*mb of a diagonal block
                        # are fully masked: skip them in scores/exp/mask/av
                        c0 = 128 * mb if mb >= 0 else 0
                        isl = slice(iw * 512, (iw + 1) * 512)
                        itsl = slice(iw * 512 + c0, (iw + 1) * 512)
                        for h in range(HPC):
                            hb = D * h
                            if jc == 0:
                                psyE[h] = psearly.tile(
                                    [D + 1, 512], f32, tag=f"psyE{h}",
                                    name="psyE")
                            pssE = psearly.tile([P, 512], f32,
                                                tag=f"pssE{h}", name="pssE")
                            nc.tensor.matmul(pssE[:, c0:], kT[hb:hb + D, jsl],
                                             qT[hb:hb + D, itsl],
                                             start=True, stop=True)
                            attE = attp.tile([P, 512], b16, tag=f"attE{h}",
                                             name="attE")
                            nc.scalar.activation(attE[:, c0:], pssE[:, c0:],
                                                 EXP, scale=0.125)
                            if mb >= 0:
                                nc.vector.tensor_tensor(
                                    attE[:, c0:], attE[:, c0:],
                                    msk_sb[:, mb, c0:], MUL)
                            va = vaug[:, jc, (D + 1) * h:(D + 1) * h + D + 1]
                            nc.tensor.matmul(psyE[h][:, c0:], va,
                                             attE[:, c0:],
                                             start=(jc == 0),
                                             stop=(jc == njc - 1),
                                             skip_group_check=True)
                        if jc == njc - 1:
                            for h in range(HPC):
                                rec = smallp.tile([1, 512], f32, tag="rec",
                                                  name="rec")
                                nc.vector.reciprocal(rec, psyE[h][D:D + 1, :])
                                recb = smallp.tile([D, 512], f32, tag="recb",
                                                   name="recb")
                                nc.gpsimd.partition_broadcast(recb, rec)
                                nc.vector.tensor_tensor(
                                    yT[D * h:D * h + D, isl],
                                    psyE[h][0:D, :], recb, MUL)
                        if jc == njc - 1 and iw == min(4, nt) - 1:
                            # last phase-1 window done: kick off the first
                            # o-projections on the freed score banks while
                            # the 1024-wide phase spins up; ACT is idle at
                            # this boundary, so it takes half the drain
                            for tb in range(6):
                                ob = attp.tile([P, 1024], b16, tag="ob",
                                               name="ob")
                                for mc in range(C // 512):
                                    pso = psearly.tile(
                                        [P, 512], f32,
                                        tag=f"pssE{(tb + mc) % 2}",
                                        name="pso")
                                    nc.tensor.matmul(
                                        pso, yT[:, tb * P:(tb + 1) * P],
                                        wo_sb[:, mc * 512:(mc + 1) * 512],
                                        start=True, stop=True)
                                    if mc == 1:
                                        nc.scalar.copy(
                                            ob[:, mc * 512:(mc + 1) * 512],
                                            pso)
                                    else:
                                        nc.vector.tensor_copy(
                                            ob[:, mc * 512:(mc + 1) * 512],
                                            pso)
                                nc.sync.dma_start(
                                    out_d[tb * P:(tb + 1) * P, :], ob)

                def qkv_chunk(tch):
                    tsl = slice(tch * 512, (tch + 1) * 512)
                    xt = xts.pop(tch)
                    if tch + 2 < nt and tch + 2 not in xts and tch >= 1:
                        x_load(tch + 2)
                    pss_qkv = {}
                    for name, w_sb in (("q", wq_sb), ("k", wk_sb), ("v", wv_sb)):
                        ps = psqkv.tile([P, 512], f32, tag=f"ps_{name}",
                                        name=f"ps_{name}")
                        for ci in range(C // P):
                            nc.tensor.matmul(
                                ps, w_sb[:, ci], xt[:, ci],
                                start=(ci == 0), stop=(ci == C // P - 1),
                            )
                        pss_qkv[name] = ps
                        emit_items(1)
                    qks = {}
                    for name in ("q", "k"):
                        qf = ropep.tile([P, 512], b16, tag=f"qf_{name}",
                                        name="qf")
                        nc.scalar.copy(qf, pss_qkv[name])
                        # rope pair-swap as a PE permutation matmul; reuses
                        # the projection's psum bank (its lifetime ended at
                        # the qf drain above)
                        sw_ps = psqkv.tile([P, 512], f32, tag=f"ps_{name}",
                                           name="sw_ps")
                        nc.tensor.matmul(sw_ps, psw_sb, qf,
                                         start=True, stop=True)
                        qks[name] = (qf, sw_ps)
                    t1s = {}
                    for name in ("q", "k"):
                        t1 = ropep.tile([P, 512], b16, tag=f"t1_{name}",
                                        name="t1")
                        nc.vector.tensor_tensor(t1, qks[name][0],
                                                cos_sb[:, tsl], MUL)
                        t1s[name] = t1
                    t2s = {}
                    for name in ("q", "k"):
                        t2 = ropep.tile([P, 512], b16, tag=f"t2_{name}",
                                        name="t2")
                        nc.vector.tensor_tensor(t2, qks[name][1],
                                                sin_sb[:, tsl], MUL)
                        t2s[name] = t2
                    for name, dest in (("q", qT), ("k", kT)):
                        nc.vector.tensor_add(dest[:, tsl], t1s[name], t2s[name])
                    vt = ropep.tile([P, 512], b16, tag="vt", name="vt")
                    nc.vector.tensor_copy(vt, pss_qkv["v"])
                    vts[tch] = vt
                    v_transposes(tch)
                    if tch < 4:
                        push_window(tch)
                    emit_items(2)

                for tch in range(nt):
                    qkv_chunk(tch)
                emit_items(len(win_items))

            # ---- phase 2: attention, 1024-wide query windows ----
            # scoresT[j,i] per (head, jc); exp on ACT (psum->sbuf, scale=1/8);
            # diagonal tiles masked; att@v accumulates y + denominator (ones
            # column of vaug). As soon as a sub-window's accumulation is done
            # (sub0 at jc=8*icg+3), it is normalized and its o-projection is
            # emitted, reusing the freed psy bank slots -- this overlaps the
            # boundary work with the rest of the window.
            with tc.tile_pool(name="psatt", bufs=1, space="PSUM") as psatt:
                def oproj_tb(tb, sub, act_copy=False):
                    # o-projection of one 128-row block of yT; the two mc
                    # halves land in two 1-bank psum tiles, drain into one
                    # [128, 1024] bf16 sbuf tile, and leave in a single DMA.
                    # act_copy splits the drain across DVE+ACT: only for
                    # window-boundary slots where ACT is otherwise idle.
                    ob = attp.tile([P, 1024], b16, tag="ob", name="ob")
                    for mc in range(C // 512):
                        pso = psatt.tile([P, 512], f32,
                                         tag=f"psy{tb % 2}{sub}",
                                         name="pso")
                        nc.tensor.matmul(
                            pso, yT[:, tb * P:(tb + 1) * P],
                            wo_sb[:, mc * 512:(mc + 1) * 512],
                            start=True, stop=True,
                        )
                        if act_copy and mc == 1:
                            nc.scalar.copy(ob[:, mc * 512:(mc + 1) * 512],
                                           pso)
                        else:
                            nc.vector.tensor_copy(
                                ob[:, mc * 512:(mc + 1) * 512], pso)
                    dma_eng = nc.scalar if (act_copy and tb % 2) else nc.sync
                    dma_eng.dma_start(out_d[tb * P:(tb + 1) * P, :], ob)

                def norm_and_oproj(icg, sub, psys, extra_tbs=()):
                    for h in range(HPC):
                        isl = slice(icg * 1024 + sub * 512,
                                    icg * 1024 + sub * 512 + 512)
                        rec = smallp.tile([1, 512], f32, tag=f"rec{h}")
                        nc.vector.reciprocal(rec, psys[h, sub][D:D + 1, :])
                        recb = smallp.tile([D, 512], f32, tag=f"recb{h}")
                        nc.gpsimd.partition_broadcast(recb, rec)
                        nc.vector.tensor_tensor(
                            yT[D * h:D * h + D, isl],
                            psys[h, sub][0:D, :], recb, MUL,
                        )
                    # sub==1 slots sit at window boundaries where ACT idles:
                    # let the drain copies use it there
                    for tb in range(icg * 8 + sub * 4, icg * 8 + sub * 4 + 4):
                        oproj_tb(tb, sub, act_copy=(sub == 1))
                    # backlog o-projections for the phase-1 windows' rows,
                    # drained here where freed psy banks + jc-loop slack exist
                    for tb in extra_tbs:
                        oproj_tb(tb, sub, act_copy=(sub == 1))

                for icg in range(2, nw):
                    # psys allocated lazily (at first att@v use) so the score
                    # tiles' first-use claims the earliest-freed psum banks
                    # at the phase handoff
                    psys = {}
                    pend = []

                    def emit_av(item, icg=icg, psys=psys):
                        h, jc, c0, att = item
                        for sub in range(2):
                            last_jc = 8 * icg + 4 * (sub + 1) - 1
                            if jc > last_jc:
                                continue
                            if (h, sub) not in psys:
                                psys[h, sub] = psatt.tile(
                                    [D + 1, 512], f32, tag=f"psy{h}{sub}",
                                    name=f"psy{h}{sub}")
                            lo = max(c0, sub * 512)
                            va = vaug[:, jc, (D + 1) * h:(D + 1) * h + D + 1]
                            nc.tensor.matmul(
                                psys[h, sub][:, lo - sub * 512:],
                                va, att[:, lo:(sub + 1) * 512],
                                start=(jc == 0), stop=(jc == last_jc),
                                skip_group_check=True,
                            )

                    njc = 8 * icg + 8
                    for jc in range(njc):
                        for h in range(HPC):
                            hb = D * h
                            jsl = slice(jc * P, (jc + 1) * P)
                            m = jc - 8 * icg
                            # window-local columns < 128*m of a diagonal
                            # block are fully masked: skip them everywhere
                            c0 = 128 * m if m >= 0 else 0
                            pss = psatt.tile([P, 1024], f32, tag=f"pss{h}",
                                             name="pss")
                            for sub in range(2):
                                if jc > 8 * icg + 4 * (sub + 1) - 1:
                                    continue  # sub-window fully above diagonal
                                lo = max(c0, sub * 512)
                                isl = slice(icg * 1024 + lo,
                                            icg * 1024 + sub * 512 + 512)
                                nc.tensor.matmul(
                                    pss[:, lo:(sub + 1) * 512],
                                    kT[hb:hb + D, jsl], qT[hb:hb + D, isl],
                                    start=True, stop=True,
                                )
                            att = attp.tile([P, 1024], b16, tag=f"att{h}",
                                            name="att")
                            nc.scalar.activation(att[:, c0:], pss[:, c0:], EXP,
                                                 scale=0.125)
                            if m >= 0:
                                hi = 512 * (m // 4) + 512
                                nc.vector.tensor_tensor(
                                    att[:, c0:hi], att[:, c0:hi],
                                    msk_sb[:, m % 4, c0 - 512 * (m // 4):],
                                    MUL,
                                )
                            pend.append((h, jc, c0, att))
                        # software pipeline: emit att@v one jc behind the
                        # scores, so an av waiting on its exp never blocks
                        # the next jc's score matmuls in the in-order PE
                        # queue -- the ACT engine stays saturated.
                        while len(pend) > 2 * HPC:
                            emit_av(pend.pop(0))
                        if jc == 8 * icg + 5:
                            norm_and_oproj(icg, 0, psys,
                                           extra_tbs=range(6, 10) if icg == 2
                                           else ())
                    for item in pend:
                        emit_av(item)
                    pend.clear()
                    norm_and_oproj(icg, 1, psys,
                                   extra_tbs=range(10, 16) if icg == 2 else ())

    nc.compile()
    return nc


def _perm_deinterleave():
    """Row permutation for Wq/Wk: per head, even rows then odd rows."""
    perm = []
    for h in range(H):
        base = h * D
        perm += [base + 2 * k for k in range(D // 2)]
        perm += [base + 2 * k + 1 for k in range(D // 2)]
    return np.array(perm)


def make_core_inputs(x, freqs_cos, freqs_sin, Wq, Wk, Wv, Wo, t=T):
    """Host-side sharding/layout prep. Returns per-core input dicts."""
    x = np.asarray(x, np.float32).reshape(t, C)
    fc = np.asarray(freqs_cos, np.float32)
    fs = np.asarray(freqs_sin, np.float32)
    Wq = np.asarray(Wq, np.float32)
    Wk = np.asarray(Wk, np.float32)
    Wv = np.asarray(Wv, np.float32)
    Wo = np.asarray(Wo, np.float32)

    xt = np.ascontiguousarray(x.T).astype(bf16)                  # [C, t]
    perm = _perm_deinterleave()
    Wq_p, Wk_p = Wq[perm], Wk[perm]

    # rope factor tables in the de-interleaved [dd, t] layout
    kidx = np.arange(P) % 32
    sgn = np.where((np.arange(P) // 32) % 2 == 0, -1.0, 1.0).astype(np.float32)
    cosb = fc.T[kidx].astype(bf16)                               # [128, t]
    sinb = (fs.T[kidx] * sgn[:, None]).astype(bf16)

    # diagonal-tile causal masks: mask4[j, m, i] = 1 iff 128*m + j <= i
    jj = np.arange(P)[:, None, None]
    mm = np.arange(4)[None, :, None]
    ii = np.arange(512)[None, None, :]
    mask4 = ((P * mm + jj) <= ii).astype(bf16)

    # rope pair-swap as a permutation matrix (0-31<->32-63, 64-95<->96-127)
    swp = np.arange(P).reshape(4, 32)[[1, 0, 3, 2]].reshape(P)
    permsw = np.zeros((P, P), np.float32)
    permsw[swp, np.arange(P)] = 1.0
    permsw = permsw.astype(bf16)

    def wlayout(w, dt=bf16, scale=1.0):
        # [C, DD] -> [P, C//P, DD] so the device load is contiguous
        return np.ascontiguousarray(
            (w * scale).reshape(C // P, P, DD).transpose(1, 0, 2)).astype(dt)

    in_maps = []
    for c in range(NCORES):
        rows = slice(c * DD, (c + 1) * DD)
        in_maps.append({
            "xt": xt,
            "permsw": permsw,
            "wq": wlayout(Wq_p[rows].T),
            "wk": wlayout(Wk_p[rows].T),
            "wv": wlayout(Wv[rows].T),
            "wo": np.ascontiguousarray(Wo[:, rows].T).astype(bf16),
            "cosb": cosb,
            "sinb": sinb,
            "mask4": mask4,
        })
    return in_maps


def run(inputs, trace=False):
    """Compile once, run on 8 cores, host-sum partials. Returns (out, results)."""
    import sys
    if "/opt/trn_rl_repo" not in sys.path:
        sys.path.insert(0, "/opt/trn_rl_repo")
    from concourse.bass_utils import run_bass_kernel_spmd

    if "nc" not in _nc_cache:
        _nc_cache["nc"] = _build_nc()
    nc = _nc_cache["nc"]

    in_maps = make_core_inputs(**inputs)
    res = run_bass_kernel_spmd(nc, in_maps, core_ids=list(range(NCORES)),
                               trace=trace)
    out = np.zeros((T, C), np.float32)
    for r in res.results:
        out += r["opart"].astype(np.float32)
    return out.reshape(1, T, C), res


def kernel(**inputs):
    import sys
    if "/opt/trn_rl_repo" not in sys.path:
        sys.path.insert(0, "/opt/trn_rl_repo")
    out, _ = run(inputs)
    return out

